# revision 1
# baseline (speedup 1.0000x reference)
"""DiffAttn Trainium2 kernel (8-core SPMD, no collectives) — v2.

Problem: B=2, T=2048, IN_DIM=OUT_DIM=1024, H=8 v-heads (2D=64), 2H=16 qk-heads
(D=32).  Core c = 4*b + g handles batch b, head-group g: qk-heads {4g..4g+3},
v-heads {2g, 2g+1}, all T queries of its batch.  Out-projection row-sharded;
host sums 4 partials per batch.

Design (measured ~240us vs 316us baseline): the Activation engine's exp
stream (16.8M exps/core, ~155us busy) is the pacing floor; everything else
hides under or beside it.

Phase B per key-tile: 4 dots matmuls (K=32 row-bands at tile_position
(32h,0), 4 distinct PSUM banks -> concurrent on the PE's 32x32 sub-arrays),
one exp instruction of [128,1024] per v-head pair, attn@v as 2x2 matmuls of
N=512 into packed [65,1024] acc tiles (a1|a2 side by side, softmax
denominator row free at partition 64 via a ones-column in v).  attn@v issue
lags LAG key-tiles behind dots/exp (pend queue) so cross-qb PSUM recycling
and boundary drains never stall the in-order PE; "e" tiles (exp outputs)
are the elastic buffer (14 bufs).

PSUM budget (8 banks exactly): "d" tag 2x[128,1024] (dots/exp double
buffer, also time-shared by phase-C matmul outputs with None spacers so
their SBUF drains never gate the dots rotation) + accA/accB [65,1024].

Phase A is mostly deferred INTO phase B: the prefix computes only warmup
(PE clock ramp during the x DMA), tb0 projections and tb0 v-tiles; the
other k/q/v projections and PE-transposes pop from a step-scheduled work
queue inside the main loop, filling early-qb PE slack (first exp at ~18us
instead of ~50us).

Phase C per query-block, lagged one qb, spread 1 chunk/kt: accs drain to
SBUF (DVE), denominator rows fanned to partition 0 via SBUF->SBUF DMA,
1/s2 via reciprocal_approx_fast (DVE), r/rinv broadcasts via gpsimd
partition_broadcast, rsqrt(ms) = exp(-0.5*ln(ms)) on the Act engine
(Ln/Exp/Square co-reside in act table 6 -> zero table swaps, loaded once
explicitly), comb packed [128,512] (vh1 lane-shifted via SBUF->SBUF DMA),
K=128 out-projection, outputs staged and DMA'd per 2-oc pair.
"""
import math

import numpy as np

H = 8
D = 32
LAMBDA_INIT = 0.8 - 0.6 * math.exp(-0.3)
B, T, IN_DIM, OUT_DIM = 2, 2048, 1024, 1024
E = 2 * H * D  # 512

N_CORES = 8
GROUPS = 4            # head groups (cores per batch)
QB = 512              # query block
NQB = T // QB         # 4
KT = 128              # key tile (PSUM partition dim for dots)
NKT = T // KT         # 16
NIN = IN_DIM // 128   # 8
LAG = 3               # attn@v issue lag in key-tiles

_compiled = None


def _build():
    import concourse.bass as bass  # noqa: F401
    import concourse.mybir as mybir
    from concourse import bacc
    from concourse.tile import TileContext

    f32 = mybir.dt.float32
    f32r = mybir.dt.float32r
    bf16 = mybir.dt.bfloat16
    AF = mybir.ActivationFunctionType
    MUL = mybir.AluOpType.mult

    nc = bacc.Bacc("TRN2", target_bir_lowering=False, num_devices=N_CORES)

    xt = nc.dram_tensor("xt", [4 * 128, NIN * 512], bf16, kind="ExternalInput")
    wq = nc.dram_tensor("wq", [128, NIN * 128], bf16, kind="ExternalInput")
    wk = nc.dram_tensor("wk", [128, NIN * 128], bf16, kind="ExternalInput")
    wv = nc.dram_tensor("wv", [128, NIN * 128], bf16, kind="ExternalInput")
    wo = nc.dram_tensor("wo", [128, OUT_DIM], bf16, kind="ExternalInput")
    lam2 = nc.dram_tensor("lam2", [2, 1], f32, kind="ExternalInput")
    gamp = nc.dram_tensor("gamp", [128, 1], f32, kind="ExternalInput")
    ones2 = nc.dram_tensor("ones2", [128, 2], bf16, kind="ExternalInput")
    idn = nc.dram_tensor("idn", [128, 128], bf16, kind="ExternalInput")
    outT = nc.dram_tensor("outT", [4 * 128, 8 * 512], f32, kind="ExternalOutput")

    with TileContext(nc) as tc:
        with tc.tile_pool(name="persist", bufs=1) as pp:
            # ---- persistent SBUF ----
            wq_sb = pp.tile([128, NIN * 128], bf16)
            wk_sb = pp.tile([128, NIN * 128], bf16)
            wv_sb = pp.tile([128, NIN * 128], bf16)
            wo_sb = pp.tile([128, OUT_DIM], bf16)
            xT_sb = pp.tile([128, 4, NIN * 512], bf16)
            # per-token-block tiles: deferred projections write only their
            # own block, so dots/attn@v never pick up false whole-tile deps
            qT_tb = [pp.tile([128, 512], bf16, name=f"qT{t}") for t in range(4)]
            kT_tb = [pp.tile([128, 512], bf16, name=f"kT{t}") for t in range(4)]
            vT_tb = [pp.tile([128, 512], bf16, name=f"vT{t}") for t in range(4)]
            v_tb = [pp.tile([128, 4, 130], bf16, name=f"v{t}") for t in range(4)]
            lam_sb = pp.tile([2, 1], f32)
            gam_sb = pp.tile([128, 1], f32)
            ones2_sb = pp.tile([128, 2], bf16)
            idn_sb = pp.tile([128, 128], bf16)
            warm = pp.tile([128, 512], bf16)
            warm_e = pp.tile([128, 512], bf16)

            nc.sync.dma_start(out=lam_sb[:, :], in_=lam2[:, :])
            nc.sync.dma_start(out=gam_sb[:, :], in_=gamp[:, :])
            nc.sync.dma_start(out=ones2_sb[:, :], in_=ones2[:, :])
            nc.sync.dma_start(out=idn_sb[:, :], in_=idn[:, :])
            nc.sync.dma_start(out=wq_sb[:, :], in_=wq[:, :])
            nc.sync.dma_start(out=wk_sb[:, :], in_=wk[:, :])
            nc.sync.dma_start(out=wv_sb[:, :], in_=wv[:, :])
            nc.sync.dma_start(out=wo_sb[:, :], in_=wo[:, :])
            for tb in range(4):
                nc.sync.dma_start(out=xT_sb[:, tb, :],
                                  in_=xt[128 * tb:128 * (tb + 1), :])

            # ---- phase A: warmup + projections ----
            with tc.tile_pool(name="psA", bufs=1, space="PSUM") as psA:
                nc.vector.memset(warm[:, :], 0.0)
                wm = psA.tile([128, 512], f32, tag="warm", bufs=1)
                for _ in range(40):
                    nc.tensor.matmul(wm[:, :], warm[:, :128], warm[:, :],
                                     start=True, stop=True)
                # natural_log_exp_and_others holds Exp+Square+Ln: one load
                # up front keeps the Act table warm for the whole kernel
                nc.scalar.add_instruction(mybir.InstLoadActFuncSet(
                    name=nc.scalar.bass.get_next_instruction_name(),
                    ins=[], outs=[], act_func_set_id=6))
                nc.scalar.activation(warm_e[:, :], wm[:, :], AF.Exp)

                # minimal prefix: only what exp(qb0, kt0..3) and the first
                # attn@v tiles need (tb0 projections + tb0 v tiles); the
                # rest is deferred into phase B's PE slack via d-tag tiles
                for dst, w_sb in ((kT_tb[0], wk_sb), (qT_tb[0], wq_sb),
                                  (vT_tb[0], wv_sb)):
                    p = psA.tile([128, 512], f32, tag="proj", bufs=3)
                    for c in range(NIN):
                        nc.tensor.matmul(
                            p[:, :], w_sb[:, 128 * c:128 * (c + 1)],
                            xT_sb[:, 0, 512 * c:512 * (c + 1)],
                            start=(c == 0), stop=(c == NIN - 1))
                    nc.vector.tensor_copy(dst[:, :], p[:, :])
                for j in range(4):
                    tp = psA.tile([128, 128], bf16, tag="vtr", bufs=2)
                    nc.tensor.transpose(
                        tp[:, :], vT_tb[0][:, 128 * j:128 * (j + 1)],
                        idn_sb[:, :])
                    nc.vector.tensor_copy(v_tb[0][:, j, 0:64], tp[:, 0:64])
                    nc.vector.tensor_copy(v_tb[0][:, j, 65:129], tp[:, 64:128])
                with tc.tile_pool(name="onescr", bufs=1) as op_:
                    oscr = op_.tile([128, 4], f32)
                    nc.vector.memset(oscr[:, :], 1.0)
                    for t in range(4):
                        nc.vector.tensor_copy(
                            v_tb[t][:, :, 64:65].rearrange("p n 1 -> p n"),
                            oscr[:, :])
                        nc.vector.tensor_copy(
                            v_tb[t][:, :, 129:130].rearrange("p n 1 -> p n"),
                            oscr[:, :])

            # ---- phase B + C interleaved ----
            with (
                tc.tile_pool(name="mp", bufs=1, space="PSUM") as mp,
                tc.tile_pool(name="ep", bufs=1) as ep,
                tc.tile_pool(name="cp", bufs=1) as cp,
            ):
                es = {}
                accs = {}
                pend_attnv = []   # (qb, kt) awaiting attn@v issue
                chunks = []       # pending phase-C closures
                hold = [0, 0]     # [skip-pops pending, permanent lag growth]

                def a_proj(dst, w_sb, tb, nm=[0]):
                    nm[0] += 1
                    pname = f"aproj{nm[0]}"
                    def f():
                        p = mp.tile([128, 1024], f32, tag="d",
                                    name=pname, bufs=2)
                        for c in range(NIN):
                            nc.tensor.matmul(
                                p[:, 0:512], w_sb[:, 128 * c:128 * (c + 1)],
                                xT_sb[:, tb, 512 * c:512 * (c + 1)],
                                start=(c == 0), stop=(c == NIN - 1))
                        nc.vector.tensor_copy(dst[tb][:, :], p[:, 0:512])
                    return f

                def a_transp(tb):
                    def f():
                        tp = mp.tile([128, 2048], bf16, tag="d",
                                     name=f"atr{tb}", bufs=2)
                        for j in range(4):
                            nc.tensor.transpose(
                                tp[:, 128 * j:128 * (j + 1)],
                                vT_tb[tb][:, 128 * j:128 * (j + 1)],
                                idn_sb[:, :])
                        for j in range(4):
                            nc.vector.tensor_copy(
                                v_tb[tb][:, j, 0:64],
                                tp[:, 128 * j:128 * j + 64])
                            nc.vector.tensor_copy(
                                v_tb[tb][:, j, 65:129],
                                tp[:, 128 * j + 64:128 * (j + 1)])
                    return f

                awork = {
                    0: a_proj(kT_tb, wk_sb, 1), 1: a_proj(kT_tb, wk_sb, 2),
                    2: a_proj(kT_tb, wk_sb, 3), 3: a_proj(vT_tb, wv_sb, 1),
                    5: a_transp(1), 6: a_proj(vT_tb, wv_sb, 2),
                    8: a_transp(2), 9: a_proj(vT_tb, wv_sb, 3),
                    10: a_proj(qT_tb, wq_sb, 1), 11: a_transp(3),
                    24: a_proj(qT_tb, wq_sb, 2), 40: a_proj(qT_tb, wq_sb, 3),
                }

                def issue_attnv(qb, kt):
                    if kt == 0:
                        accs[(qb, 0)] = mp.tile([65, 1024], f32, tag="accA",
                                                name=f"accA{qb}", bufs=1)
                        accs[(qb, 1)] = mp.tile([65, 1024], f32, tag="accB",
                                                name=f"accB{qb}", bufs=1)
                    for vh in range(2):
                        e = es.pop((qb, kt, vh))
                        for hh in range(2):
                            nc.tensor.matmul(
                                accs[(qb, vh)][:, 512 * hh:512 * (hh + 1)],
                                v_tb[kt // 4][:, kt % 4, 65 * vh:65 * (vh + 1)],
                                e[:, 512 * hh:512 * (hh + 1)],
                                start=(kt == 0), stop=(kt == NKT - 1))

                def make_chunks(qb):
                    aA, aB = accs.pop((qb, 0)), accs.pop((qb, 1))
                    sA = cp.tile([1, 1024], f32, tag="sA", name=f"sA{qb}",
                                 bufs=2)
                    sB = cp.tile([1, 1024], f32, tag="sB", name=f"sB{qb}",
                                 bufs=2)
                    sr = cp.tile([65, 2048], f32, tag="sr", name=f"sr{qb}",
                                 bufs=1)
                    a_sb0 = cp.tile([64, 1024], f32, tag="asb0",
                                    name=f"asb0_{qb}", bufs=2)
                    a_sb1 = cp.tile([64, 1024], f32, tag="asb1",
                                    name=f"asb1_{qb}", bufs=2)
                    # boundary: drain accs to SBUF (lane-aligned copies split
                    # across DVE and GpSimd so they run in parallel), then
                    # fan the denominator rows onto partition 0 via DMA
                    nc.vector.tensor_copy(sr[64:65, 0:1024], aA[64:65, :])
                    nc.vector.tensor_copy(sr[64:65, 1024:2048], aB[64:65, :])
                    nc.vector.tensor_copy(a_sb0[:, :], aA[0:64, :])
                    nc.vector.tensor_copy(a_sb1[:, :], aB[0:64, :])
                    # sA/sB: partition 0 = [s1|s2] for vh0 / vh1
                    nc.sync.dma_start(out=sA[0:1, :], in_=sr[64:65, 0:1024])
                    nc.sync.dma_start(out=sB[0:1, :], in_=sr[64:65, 1024:2048])

                    st = {}

                    def c_ratio():
                        rcp = cp.tile([1, 1024], f32, tag="rcp", bufs=2)
                        nc.vector.reciprocal_approx_fast(
                            out=rcp[0:1, 0:512], in_=sA[0:1, 512:1024])
                        nc.vector.reciprocal_approx_fast(
                            out=rcp[0:1, 512:1024], in_=sB[0:1, 512:1024])
                        r2 = cp.tile([1, 1024], f32, tag="r2", bufs=2)
                        nc.vector.scalar_tensor_tensor(
                            r2[0:1, 0:512], sA[0:1, 0:512], lam_sb[0:1, 0:1],
                            rcp[0:1, 0:512], op0=MUL, op1=MUL)
                        nc.vector.scalar_tensor_tensor(
                            r2[0:1, 512:1024], sB[0:1, 0:512], lam_sb[0:1, 0:1],
                            rcp[0:1, 512:1024], op0=MUL, op1=MUL)
                        st["r2"] = r2

                    def c_rb():
                        rb = cp.tile([64, 1024], f32, tag="rb",
                                     name=f"rb{qb}", bufs=2)
                        nc.gpsimd.partition_broadcast(
                            rb[:, 0:512], st["r2"][0:1, 0:512], channels=64)
                        nc.gpsimd.partition_broadcast(
                            rb[:, 512:1024], st["r2"][0:1, 512:1024],
                            channels=64)
                        st["rb"] = rb

                    def c_comb0():
                        t2 = cp.tile([64, 512], f32, tag="t2a", bufs=1)
                        comb = cp.tile([128, 512], f32, tag="comb",
                                       name=f"comb{qb}", bufs=2)
                        nc.vector.tensor_mul(t2[:, :], a_sb0[:, 512:1024],
                                             st["rb"][:, 0:512])
                        nc.vector.tensor_sub(comb[0:64, :], a_sb0[:, 0:512],
                                             t2[:, :])
                        st["comb"] = comb

                    def c_comb1():
                        t2 = cp.tile([64, 512], f32, tag="t2b", bufs=1)
                        ctmp = cp.tile([64, 512], f32, tag="ctmp", bufs=1)
                        nc.vector.tensor_mul(t2[:, :], a_sb1[:, 512:1024],
                                             st["rb"][:, 512:1024])
                        nc.vector.tensor_sub(ctmp[:, :], a_sb1[:, 0:512],
                                             t2[:, :])
                        nc.sync.dma_start(out=st["comb"][64:128, :],
                                          in_=ctmp[:, :])

                    def c_sq():
                        sq = cp.tile([128, 512], bf16, tag="sq", bufs=2)
                        nc.scalar.activation(sq[:, :], st["comb"][:, :],
                                             AF.Square)
                        st["sq"] = sq

                    def c_ss():
                        sst = mp.tile([128, 1024], f32, tag="d",
                                      name=f"ss{qb}", bufs=2)
                        nc.tensor.matmul(sst[0:1, 0:512], ones2_sb[:, 0:1],
                                         st["sq"][:, :], start=True, stop=True)
                        nc.tensor.matmul(sst[0:1, 512:1024], ones2_sb[:, 1:2],
                                         st["sq"][:, :], start=True, stop=True)
                        st["ss"] = sst

                    def c_rinv():
                        rln = cp.tile([1, 1024], f32, tag="rln", bufs=2)
                        nc.scalar.activation(rln[:, :], st["ss"][0:1, 0:1024],
                                             AF.Ln, scale=1.0 / 64.0)
                        rinv = cp.tile([1, 1024], f32, tag="rinv", bufs=2)
                        nc.scalar.activation(rinv[:, :], rln[:, :], AF.Exp,
                                             scale=-0.5)
                        st["rinv"] = rinv

                    def c_rb2():
                        rb2 = cp.tile([128, 1024], f32, tag="rb2",
                                      name=f"rb2_{qb}", bufs=2)
                        nc.gpsimd.partition_broadcast(
                            rb2[:, 0:512], st["rinv"][0:1, 0:512],
                            channels=128)
                        nc.gpsimd.partition_broadcast(
                            rb2[:, 512:1024], st["rinv"][0:1, 512:1024],
                            channels=128)
                        st["rb2"] = rb2

                    def c_finl():
                        finl = cp.tile([128, 512], bf16, tag="finl",
                                       name=f"finl{qb}", bufs=2)
                        nc.vector.scalar_tensor_tensor(
                            finl[0:64, :], st["comb"][0:64, :], gam_sb[0:64, 0:1],
                            st["rb2"][0:64, 0:512], op0=MUL, op1=MUL)
                        nc.vector.scalar_tensor_tensor(
                            finl[64:128, :], st["comb"][64:128, :],
                            gam_sb[64:128, 0:1], st["rb2"][64:128, 512:1024],
                            op0=MUL, op1=MUL)
                        st["finl"] = finl

                    def c_opj(p):
                        def f():
                            opj = mp.tile([128, 1024], f32, tag="d",
                                          name=f"opj{qb}_{p}", bufs=2)
                            for j in range(2):
                                oc = 2 * p + j
                                nc.tensor.matmul(
                                    opj[:, 512 * j:512 * (j + 1)],
                                    wo_sb[:, 128 * oc:128 * (oc + 1)],
                                    st["finl"][:, :], start=True, stop=True)
                            ostg = cp.tile([128, 1024], f32, tag="ostg",
                                           name=f"ostg{qb}_{p}", bufs=2)
                            nc.vector.tensor_copy(ostg[:, :], opj[:, :])
                            nc.sync.dma_start(
                                out=outT[128 * qb:128 * (qb + 1),
                                         1024 * p:1024 * (p + 1)],
                                in_=ostg[:, :])
                        return f

                    return [c_ratio, c_rb, c_comb0, c_comb1, c_sq, c_ss,
                            c_rinv, c_rb2, c_finl,
                            c_opj(0), None, c_opj(1), None,
                            c_opj(2), None, c_opj(3)]

                for qb in range(NQB):
                    qs = qb * QB
                    for kt in range(NKT):
                        # all four dots back-to-back into 4 distinct PSUM
                        # banks -> 4-band row-tile concurrency on the PE
                        ds = [mp.tile([128, 1024], f32, tag="d",
                                      name=f"d{qb}_{kt}_{vh}", bufs=2)
                              for vh in range(2)]
                        for h in range(4):
                            nc.tensor.matmul(
                                ds[h // 2][:, 512 * (h % 2):512 * (h % 2 + 1)],
                                kT_tb[kt // 4][32 * h:32 * (h + 1),
                                               KT * (kt % 4):KT * (kt % 4 + 1)],
                                qT_tb[qb][32 * h:32 * (h + 1), :],
                                start=True, stop=True,
                                tile_position=(32 * h, 0))
                        for vh in range(2):
                            e = ep.tile([128, 1024], bf16, tag="e",
                                        name=f"e{qb}_{kt}_{vh}", bufs=14)
                            nc.scalar.activation(e[:, :], ds[vh][:, :], AF.Exp)
                            es[(qb, kt, vh)] = e
                        pend_attnv.append((qb, kt))
                        if hold[0]:
                            hold[0] -= 1
                        else:
                            while len(pend_attnv) > LAG:
                                aqb, akt = pend_attnv.pop(0)
                                issue_attnv(aqb, akt)
                                if akt == NKT - 1:
                                    chunks.extend(make_chunks(aqb))
                                    # give the boundary drain one extra kt
                                    # before the next qb's first attn@v
                                    # needs the acc banks back
                                    hold[0] = 1
                                    break
                        if chunks:
                            ck = chunks.pop(0)
                            if ck is not None:
                                ck()
                        aw = awork.pop(16 * qb + kt, None)
                        if aw is not None:
                            aw()
                # drain
                while pend_attnv:
                    aqb, akt = pend_attnv.pop(0)
                    issue_attnv(aqb, akt)
                    if akt == NKT - 1:
                        chunks.extend(make_chunks(aqb))
                while chunks:
                    ck = chunks.pop(0)
                    if ck is not None:
                        ck()

    nc.compile()
    return nc


def _get_compiled():
    global _compiled
    if _compiled is None:
        _compiled = _build()
    return _compiled


def make_in_maps(x, Wq, Wkv, Wout, lambda_q1, lambda_k1, lambda_q2, lambda_k2,
                 gamma):
    import ml_dtypes
    bf = ml_dtypes.bfloat16
    x = np.asarray(x, dtype=np.float32)
    Wq = np.asarray(Wq, dtype=np.float32)
    Wkv = np.asarray(Wkv, dtype=np.float32)
    Wout = np.asarray(Wout, dtype=np.float32)
    lam_v = (math.exp(float(np.dot(lambda_q1, lambda_k1)))
             - math.exp(float(np.dot(lambda_q2, lambda_k2))) + LAMBDA_INIT)
    lam_arr = np.full((2, 1), lam_v, dtype=np.float32)
    gam_arr = np.tile(
        (np.asarray(gamma, dtype=np.float32) * (1.0 - LAMBDA_INIT)), 2
    ).reshape(128, 1).copy()
    o2 = np.zeros((128, 2), dtype=bf)
    o2[0:64, 0] = 1.0
    o2[64:128, 1] = 1.0
    idn = np.eye(128, dtype=np.float32).astype(bf)
    Wq_s = (Wq * (D ** -0.5)).astype(np.float32)
    Wk = Wkv[:, :E]
    Wv = Wkv[:, E:]

    def wtile(W, g):
        # [1024, 128] slice -> [128, 8*128] with [p, c*128+m] = W[c*128+p, m]
        ws = W[:, 128 * g:128 * (g + 1)]
        return np.ascontiguousarray(
            ws.reshape(8, 128, 128).transpose(1, 0, 2).reshape(128, 1024)
        ).astype(bf)

    xts = []
    for b in range(B):
        xb = x[b]  # [2048, 1024]
        a = xb.reshape(4, 512, 8, 128).transpose(0, 3, 2, 1)  # [tb,p,c,m]
        xts.append(np.ascontiguousarray(a.reshape(512, 4096)).astype(bf))

    in_maps = []
    for c in range(N_CORES):
        b, g = divmod(c, GROUPS)
        in_maps.append({
            "xt": xts[b],
            "wq": wtile(Wq_s, g),
            "wk": wtile(Wk, g),
            "wv": wtile(Wv, g),
            "wo": np.ascontiguousarray(
                Wout[128 * g:128 * (g + 1), :]).astype(bf),
            "lam2": lam_arr,
            "gamp": gam_arr,
            "ones2": o2,
            "idn": idn,
        })
    return in_maps


def kernel(x, Wq, Wkv, Wout, lambda_q1, lambda_k1, lambda_q2, lambda_k2,
           gamma, _run_kw=None):
    import sys
    if "/opt/trn_rl_repo" not in sys.path:
        sys.path.insert(0, "/opt/trn_rl_repo")
    from concourse.bass_utils import run_bass_kernel_spmd

    nc = _get_compiled()
    in_maps = make_in_maps(x, Wq, Wkv, Wout, lambda_q1, lambda_k1,
                           lambda_q2, lambda_k2, gamma)
    res = run_bass_kernel_spmd(nc, in_maps, list(range(N_CORES)),
                               **(_run_kw or {}))
    out = np.zeros((B, T, OUT_DIM), dtype=np.float32)
    for c in range(N_CORES):
        r = res.results[c]["outT"]  # [512, 4096]
        part = r.reshape(4, 128, 8, 512).transpose(0, 3, 2, 1).reshape(T, OUT_DIM)
        out[c // GROUPS] += part
    kernel.last_result = res
    return out

